# revision 19
# baseline (speedup 1.0000x reference)
"""Causal self-attention (B=2, T=2048, C=1024, H=16, D=64) on 8 TRN2 NeuronCores.

Tensor-parallel over heads: each core owns 2 heads. w_qkv columns and w_out
rows are sharded by head; x (transposed on host) is replicated. The host sums
the 8 fp16 partial outputs (the TP all-reduce) and adds b_out.

v2 layout/schedule, designed from the baseline's NTFF profile:
 - Single software-pipelined loop: step `it` emits attention for query block
   it-1 interleaved (as PE filler) with the qkv projection for t-block `it`
   and the output projection for block it-2.  The PE instruction stream stays
   dense so the HAM clock gate keeps the PE at 2.4 GHz instead of 1.2.
 - Scalar (ACT) engine runs ONLY exp.  Score tiles are [128 keys, 2, 512 q]
   pairs so each 352-cycle ACTIVATE overhead covers 1024 columns.
 - All PSUM drains (qkv bias-adds, v-transpose copies, attn normalize,
   out-proj copies) run on the vector engine.
 - Softmax normalization: denominators come free from a ones-column in V;
   reciprocal (DVE) -> partition_broadcast (gpsimd) -> fused mul (DVE),
   no DMA round-trips.
 - Diagonal (causal) tiles: matmuls compute only valid columns; masking is
   one 128x128 affine_select per triangle on gpsimd.
 - Output written fp16 (halves the out-DMA); host accumulates in fp32.
"""

import numpy as np

import concourse.bass as bass
from concourse import bacc
import concourse.bass_utils as bass_utils
import concourse.mybir as mybir
from concourse.masks import make_identity
from concourse.tile import TileContext

B, T, C, H, D = 2, 2048, 1024, 16, 64
BT = B * T
NCORES = 8
HPC = H // NCORES          # heads per core
JL = 3 * HPC * D           # 384 local qkv output columns
QB = 512                   # queries per block
KT = 128                   # keys per tile
NQB = T // QB              # 4 query blocks per batch
NTB = BT // QB             # 8 t-blocks total
F32 = mybir.dt.float32
F16 = mybir.dt.float16
AF = mybir.ActivationFunctionType
SCALE = float(D) ** -0.5

_cache = {}


def _build_bass():
    nc = bacc.Bacc("TRN2", target_bir_lowering=False, debug=False)
    xT = nc.dram_tensor("xT", [C, BT], F16, kind="ExternalInput").ap()
    wqkv = nc.dram_tensor("wqkv", [C, JL], F16, kind="ExternalInput").ap()
    bqkv = nc.dram_tensor("bqkv", [128, 3], F32, kind="ExternalInput").ap()
    wout = nc.dram_tensor("wout", [HPC * D, C], F16, kind="ExternalInput").ap()
    outp = nc.dram_tensor("outp", [BT, C], F16, kind="ExternalOutput").ap()

    with TileContext(nc) as tc:
        with (
            tc.tile_pool(name="const", bufs=1) as const,
            tc.tile_pool(name="xtp", bufs=3) as xtp,
            tc.tile_pool(name="vtp", bufs=2) as vtp,
            tc.tile_pool(name="ptp", bufs=6) as ptp,
            tc.tile_pool(name="rbp", bufs=2) as rbp,
            tc.tile_pool(name="obp", bufs=4) as obp,
            tc.tile_pool(name="psA", bufs=2, space="PSUM") as psA,
            tc.tile_pool(name="psS", bufs=2, space="PSUM") as psS,
            tc.tile_pool(name="psPV", bufs=2, space="PSUM") as psPV,
        ):
            # ---- static tensors
            w_sb = const.tile([128, 8, JL], F16)
            nc.sync.dma_start(out=w_sb, in_=wqkv.rearrange("(k p) j -> p k j", p=128))
            wout_sb = const.tile([128, C], F16)
            nc.sync.dma_start(out=wout_sb, in_=wout)
            bias_sb = const.tile([128, 3], F32)
            nc.sync.dma_start(out=bias_sb, in_=bqkv)
            ident = const.tile([128, 128], F16)
            make_identity(nc, ident)
            qT = const.tile([128, BT], F16)    # rows: [h0 d64 | h1 d64]
            kTt = const.tile([128, BT], F16)
            v_sb = const.tile([128, HPC, B, T // KT, D + 1], F16)
            for h in range(HPC):
                for b_ in range(B):
                    nc.vector.memset(v_sb[:, h, b_, :, D:D + 1], 1.0)
            attnTc = const.tile([128, BT], F16)

            xts = {}
            vts = {}

            def issue_x(tb):
                xt = xtp.tile([128, 8, QB], F16, tag="xt", name="xt")
                nc.sync.dma_start(
                    out=xt,
                    in_=xT[:, tb * QB:(tb + 1) * QB].rearrange(
                        "(k p) t -> p k t", p=128))
                xts[tb] = xt

            # ---------- qkv projection chunks for one 512-col t-block ----------
            def proj_chunks(tb):
                tcols = slice(tb * QB, (tb + 1) * QB)
                chunks = []

                def m_chunk(m):
                    def f():
                        xt = xts[tb]
                        ps = psA.tile([128, QB], F32, tag="a", name="psa")
                        for k in range(8):
                            nc.tensor.matmul(
                                ps,
                                lhsT=w_sb[:, k, m * 128:(m + 1) * 128],
                                rhs=xt[:, k, :],
                                start=(k == 0), stop=(k == 7))
                        if m == 0:
                            nc.vector.tensor_scalar_add(
                                qT[:, tcols], ps, bias_sb[:, 0:1])
                        elif m == 1:
                            nc.vector.tensor_scalar_add(
                                kTt[:, tcols], ps, bias_sb[:, 1:2])
                        else:
                            vt = vtp.tile([128, QB], F16, tag="vt", name="vt")
                            nc.vector.tensor_scalar_add(vt, ps, bias_sb[:, 2:3])
                            vts[tb] = vt
                    return f

                def tr_chunk(c4):
                    def f():
                        vt = vts[tb]
                        t0 = tb * QB + c4 * 128
                        b_, kt = t0 // T, (t0 % T) // KT
                        pst = psA.tile([128, 2, D], F16, tag="a", name="pst")
                        nc.tensor.transpose(
                            pst, vt[:, c4 * 128:(c4 + 1) * 128], ident)
                        nc.vector.tensor_copy(
                            out=v_sb[:, 0:2, b_, kt, 0:D], in_=pst)
                    return f

                for m in range(3):
                    chunks.append(m_chunk(m))
                for c4 in range(QB // 128):
                    chunks.append(tr_chunk(c4))
                return chunks

            # ---------- attention chunks for one (b, qb) ----------
            def attn_chunks(b_, qb):
                q0 = b_ * T + qb * QB
                n_kt = (qb + 1) * (QB // KT)
                pvs = {}
                pts = {}
                aus = {}
                chunks = []

                def pv_tile(h):
                    def f():
                        pvs[h] = psPV.tile([D + 1, QB], F32, tag="pv", name="pv")
                    return f

                def sc_pair(h, p):
                    def f():
                        hs = slice(h * 64, (h + 1) * 64)
                        ps = psS.tile([128, 2, QB], F32, tag="s", name="pss")
                        for j in range(2):
                            kt = 2 * p + j
                            nc.tensor.matmul(
                                ps[:, j, :],
                                lhsT=kTt[hs, b_ * T + kt * KT:
                                         b_ * T + (kt + 1) * KT],
                                rhs=qT[hs, q0:q0 + QB],
                                start=True, stop=True)
                        pt = ptp.tile([128, 2, QB], F16, tag="pt", name="pt")
                        nc.scalar.activation(
                            out=pt, in_=ps, func=AF.Exp, scale=SCALE)
                        pts[(h, p)] = pt
                    return f

                def pv_pair(h, p):
                    def f():
                        pt = pts.pop((h, p))
                        for j in range(2):
                            kt = 2 * p + j
                            nc.tensor.matmul(
                                pvs[h],
                                lhsT=v_sb[:, h, b_, kt, :],
                                rhs=pt[:, j, :],
                                start=(kt == 0), stop=False)
                    return f

                def sc_diag(h, dp):
                    def f():
                        hs = slice(h * 64, (h + 1) * 64)
                        r0 = 2 * dp
                        off0 = r0 * KT
                        ps = psS.tile([128, 2, QB], F32, tag="s", name="psd")
                        for j in range(2):
                            r = r0 + j
                            kt = qb * (QB // KT) + r
                            off = r * KT
                            nc.tensor.matmul(
                                ps[:, j, off:QB],
                                lhsT=kTt[hs, b_ * T + kt * KT:
                                         b_ * T + (kt + 1) * KT],
                                rhs=qT[hs, q0 + off:q0 + QB],
                                start=True, stop=True)
                        pt = ptp.tile([128, 2, QB], F16, tag="pt", name="ptd")
                        nc.scalar.activation(
                            out=pt[:, :, off0:QB], in_=ps[:, :, off0:QB],
                            func=AF.Exp, scale=SCALE)
                        # causal mask: the two 128-wide triangle blocks
                        for j in range(2):
                            off = (r0 + j) * KT
                            tri = pt[:, j, off:off + KT]
                            nc.gpsimd.affine_select(
                                out=tri, in_=tri,
                                compare_op=mybir.AluOpType.is_ge, fill=0.0,
                                base=0, channel_multiplier=-1,
                                pattern=[[1, KT]])
                        pts[(h, 'd', dp)] = pt
                    return f

                def pv_diag(h, dp):
                    def f():
                        pt = pts.pop((h, 'd', dp))
                        for j in range(2):
                            r = 2 * dp + j
                            kt = qb * (QB // KT) + r
                            off = r * KT
                            nc.tensor.matmul(
                                pvs[h][:, off:QB],
                                lhsT=v_sb[:, h, b_, kt, :],
                                rhs=pt[:, j, off:QB],
                                start=(kt == 0), stop=(kt == n_kt - 1))
                    return f

                def drain(h):
                    # copy pv psum (attn + den row, unnormalized) to SBUF
                    # immediately so the PSUM slot frees for the next step.
                    def f():
                        au = rbp.tile([D + 1, QB], F16, tag=f"au{h}",
                                      name="au", bufs=2)
                        nc.vector.tensor_copy(out=au, in_=pvs[h])
                        aus[h] = au
                    return f

                def norm():
                    # phase a (now): move den row to partition 0, recip,
                    # broadcast, and stage unnormalized h1 into attnTc.
                    # phase b (deferred to the next step, via pending_muls):
                    # the two DVE muls — by then the broadcast is complete,
                    # so they never head-of-line-block PSUM drains.
                    cols = slice(q0, q0 + QB)
                    rbs = {}
                    for h in range(HPC):
                        au = aus[h]
                        r0 = rbp.tile([1, QB], F16, tag="r0", name="r0")
                        nc.gpsimd.dma_start(out=r0, in_=au[D:D + 1, :])
                        rc = rbp.tile([1, QB], F32, tag="rc", name="rc")
                        nc.vector.reciprocal(out=rc, in_=r0)
                        rb = rbp.tile([128, QB], F32, tag="rb", name="rb")
                        nc.gpsimd.partition_broadcast(rb, rc)
                        rbs[h] = rb
                    nc.gpsimd.dma_start(
                        out=attnTc[64:128, cols], in_=aus[1][0:D, :])

                    def muls():
                        nc.vector.tensor_mul(
                            attnTc[0:64, cols], aus[0][0:D, :], rbs[0][0:64, :])
                        nc.vector.tensor_mul(
                            attnTc[64:128, cols], attnTc[64:128, cols],
                            rbs[1][64:128, :])
                    pending_muls.append(muls)

                chunks.append((pv_tile(0), False))
                chunks.append((pv_tile(1), False))
                for p in range(2 * qb):            # full kt pairs
                    chunks.append((sc_pair(0, p), False))
                    chunks.append((sc_pair(1, p), True))
                    chunks.append((pv_pair(0, p), False))
                    chunks.append((pv_pair(1, p), True))
                for dp in range(2):                # diagonal pairs
                    chunks.append((sc_diag(0, dp), False))
                    chunks.append((sc_diag(1, dp), True))
                    chunks.append((pv_diag(0, dp), False))
                    chunks.append((pv_diag(1, dp), True))
                chunks.append((drain(0), False))
                chunks.append((drain(1), False))
                chunks.append((norm, False))
                return chunks

            # ---------- output projection chunks for one (b, qb) ----------
            def outproj_chunks(s):
                chunks = []

                def op_chunk(tt, ch):
                    def f():
                        t0 = s * QB + tt * 128
                        po = psA.tile([128, QB], F32, tag="a", name="po")
                        nc.tensor.matmul(
                            po,
                            lhsT=attnTc[:, t0:t0 + 128],
                            rhs=wout_sb[:, ch * QB:(ch + 1) * QB],
                            start=True, stop=True)
                        ob = obp.tile([128, QB], F16, tag="ob", name="ob")
                        # split the PSUM drains across ACT and DVE
                        if (tt + ch) % 2 == 0:
                            nc.vector.tensor_copy(out=ob, in_=po)
                        else:
                            nc.scalar.copy(out=ob, in_=po)
                        nc.sync.dma_start(
                            out=outp[t0:t0 + 128, ch * QB:(ch + 1) * QB],
                            in_=ob)
                    return f

                for tt in range(QB // 128):
                    for ch in range(2):
                        chunks.append(op_chunk(tt, ch))
                return chunks

            # ---------- software-pipelined emission ----------
            issue_x(0)
            issue_x(1)
            pending_muls = []
            for it in range(NTB + 1):
                if it + 2 < NTB:
                    issue_x(it + 2)
                # proj filler goes into the FIRST part of the attention
                # stream; outproj filler into the LAST part.
                fillA = proj_chunks(it) if it < NTB else []
                fillB = outproj_chunks(it - 2) if it >= 2 else []
                if 1 <= it:
                    s = it - 1
                    att = attn_chunks(s // NQB, s % NQB)
                    natt = len(att)
                    nA, nB = len(fillA), len(fillB)
                    cut = (natt * 11) // 20
                    fa = fb = 0
                    for i, (ck, fill_ok) in enumerate(att):
                        ck()
                        if i == min(4, natt - 2) and pending_muls:
                            for mf in pending_muls:
                                mf()
                            pending_muls.clear()
                        if i < cut:
                            wantA = (nA * (i + 1) + cut - 1) // cut
                            while fa < min(wantA, nA):
                                fillA[fa]()
                                fa += 1
                        else:
                            wantB = (nB * (i + 1 - cut) + natt - cut - 1) \
                                // (natt - cut)
                            while fb < min(wantB, nB):
                                fillB[fb]()
                                fb += 1
                    while fa < nA:
                        fillA[fa]()
                        fa += 1
                    while fb < nB:
                        fillB[fb]()
                        fb += 1
                    if it == NTB:
                        # drain the final block's normalize + out-projection
                        for mf in pending_muls:
                            mf()
                        pending_muls.clear()
                        for ck in outproj_chunks(it - 1):
                            ck()
                else:
                    for ck in fillA + fillB:
                        ck()
    nc.compile()
    return nc


def _prep_in_maps(x, w_qkv, b_qkv, w_out):
    xTfull = np.ascontiguousarray(x.reshape(BT, C).T.astype(np.float16))
    in_maps = []
    for core in range(NCORES):
        hs = [core * HPC + i for i in range(HPC)]
        wq = np.ascontiguousarray(np.concatenate(
            [w_qkv[:, sec * C + h * D: sec * C + (h + 1) * D]
             for sec in range(3) for h in hs], axis=1).astype(np.float16))
        bq = np.ascontiguousarray(np.stack(
            [np.concatenate([b_qkv[sec * C + h * D: sec * C + (h + 1) * D] for h in hs])
             for sec in range(3)], axis=1))
        wo = np.ascontiguousarray(np.concatenate(
            [w_out[h * D:(h + 1) * D, :] for h in hs], axis=0).astype(np.float16))
        in_maps.append({"xT": xTfull, "wqkv": wq, "bqkv": bq, "wout": wo})
    return in_maps


LAST_RESULTS = None


def kernel(x, w_qkv, b_qkv, w_out, b_out):
    global LAST_RESULTS
    x = np.asarray(x, np.float32)
    w_qkv = np.asarray(w_qkv, np.float32)
    b_qkv = np.asarray(b_qkv, np.float32)
    w_out = np.asarray(w_out, np.float32)
    b_out = np.asarray(b_out, np.float32)

    if "nc" not in _cache:
        _cache["nc"] = _build_bass()
    nc = _cache["nc"]

    in_maps = _prep_in_maps(x, w_qkv, b_qkv, w_out)
    res = bass_utils.run_bass_kernel_spmd(nc, in_maps, core_ids=list(range(NCORES)))
    LAST_RESULTS = res

    out = res.results[0]["outp"].astype(np.float32)
    for r_ in res.results[1:]:
        out += r_["outp"].astype(np.float32)
    out += b_out
    return out.reshape(B, T, C)


# revision 22
# speedup vs baseline: 1.2168x; 1.2168x over previous
"""Causal self-attention (B=2, T=2048, C=1024, H=16, D=64) on 8 TRN2 NeuronCores.

Tensor-parallel over heads: each core owns 2 heads. w_qkv columns and w_out
rows are sharded by head; x (transposed on host) is replicated. The host sums
the 8 fp16 partial outputs (the TP all-reduce) and adds b_out.

v2 layout/schedule, designed from the baseline's NTFF profile:
 - Single software-pipelined loop: step `it` emits attention for query block
   it-1 interleaved (as PE filler) with the qkv projection for t-block `it`
   and the output projection for block it-2.  The PE instruction stream stays
   dense so the HAM clock gate keeps the PE at 2.4 GHz instead of 1.2.
 - Scalar (ACT) engine runs ONLY exp.  Score tiles are [128 keys, 2, 512 q]
   pairs so each 352-cycle ACTIVATE overhead covers 1024 columns.
 - All PSUM drains (qkv bias-adds, v-transpose copies, attn normalize,
   out-proj copies) run on the vector engine.
 - Softmax normalization: denominators come free from a ones-column in V;
   reciprocal (DVE) -> partition_broadcast (gpsimd) -> fused mul (DVE),
   no DMA round-trips.
 - Diagonal (causal) tiles: matmuls compute only valid columns; masking is
   one 128x128 affine_select per triangle on gpsimd.
 - Output written fp16 (halves the out-DMA); host accumulates in fp32.
"""

import numpy as np

import concourse.bass as bass
from concourse import bacc
import concourse.bass_utils as bass_utils
import concourse.mybir as mybir
from concourse.masks import make_identity
from concourse.tile import TileContext

B, T, C, H, D = 2, 2048, 1024, 16, 64
BT = B * T
NCORES = 8
HPC = H // NCORES          # heads per core
JL = 3 * HPC * D           # 384 local qkv output columns
QB = 512                   # queries per block
KT = 128                   # keys per tile
NQB = T // QB              # 4 query blocks per batch
NTB = BT // QB             # 8 t-blocks total
F32 = mybir.dt.float32
F16 = mybir.dt.float16
AF = mybir.ActivationFunctionType
SCALE = float(D) ** -0.5

_cache = {}


def _build_bass():
    nc = bacc.Bacc("TRN2", target_bir_lowering=False, debug=False)
    xT = nc.dram_tensor("xT", [C, BT], F16, kind="ExternalInput").ap()
    wqkv = nc.dram_tensor("wqkv", [C, JL], F16, kind="ExternalInput").ap()
    bqkv = nc.dram_tensor("bqkv", [128, 3], F32, kind="ExternalInput").ap()
    wout = nc.dram_tensor("wout", [HPC * D, C], F16, kind="ExternalInput").ap()
    outp = nc.dram_tensor("outp", [BT, C], F16, kind="ExternalOutput").ap()

    with TileContext(nc) as tc:
        with (
            tc.tile_pool(name="const", bufs=1) as const,
            tc.tile_pool(name="xtp", bufs=3) as xtp,
            tc.tile_pool(name="vtp", bufs=2) as vtp,
            tc.tile_pool(name="ptp", bufs=6) as ptp,
            tc.tile_pool(name="rbp", bufs=2) as rbp,
            tc.tile_pool(name="obp", bufs=4) as obp,
            tc.tile_pool(name="psA", bufs=2, space="PSUM") as psA,
            tc.tile_pool(name="psS", bufs=2, space="PSUM") as psS,
            tc.tile_pool(name="psPV", bufs=2, space="PSUM") as psPV,
        ):
            # ---- static tensors
            w_sb = const.tile([128, 8, JL], F16)
            nc.sync.dma_start(out=w_sb, in_=wqkv.rearrange("(k p) j -> p k j", p=128))
            wout_sb = const.tile([128, C], F16)
            nc.sync.dma_start(out=wout_sb, in_=wout)
            bias_sb = const.tile([128, 3], F32)
            nc.sync.dma_start(out=bias_sb, in_=bqkv)
            ident = const.tile([128, 128], F16)
            make_identity(nc, ident)
            qT = const.tile([128, BT], F16)    # rows: [h0 d64 | h1 d64]
            kTt = const.tile([128, BT], F16)
            v_sb = const.tile([128, HPC, B, T // KT, D + 1], F16)
            for h in range(HPC):
                for b_ in range(B):
                    nc.vector.memset(v_sb[:, h, b_, :, D:D + 1], 1.0)
            attnTc = const.tile([128, BT], F16)

            xts = {}
            vts = {}

            def issue_x(tb):
                xt = xtp.tile([128, 8, QB], F16, tag="xt", name="xt")
                nc.sync.dma_start(
                    out=xt,
                    in_=xT[:, tb * QB:(tb + 1) * QB].rearrange(
                        "(k p) t -> p k t", p=128))
                xts[tb] = xt

            # ---------- qkv projection chunks for one 512-col t-block ----------
            def proj_chunks(tb):
                tcols = slice(tb * QB, (tb + 1) * QB)
                chunks = []

                def m_chunk(m):
                    def f():
                        xt = xts[tb]
                        ps = psA.tile([128, QB], F32, tag="a", name="psa")
                        for k in range(8):
                            nc.tensor.matmul(
                                ps,
                                lhsT=w_sb[:, k, m * 128:(m + 1) * 128],
                                rhs=xt[:, k, :],
                                start=(k == 0), stop=(k == 7))
                        if m == 0:
                            nc.vector.tensor_scalar_add(
                                qT[:, tcols], ps, bias_sb[:, 0:1])
                        elif m == 1:
                            nc.vector.tensor_scalar_add(
                                kTt[:, tcols], ps, bias_sb[:, 1:2])
                        else:
                            vt = vtp.tile([128, QB], F16, tag="vt", name="vt")
                            nc.vector.tensor_scalar_add(vt, ps, bias_sb[:, 2:3])
                            vts[tb] = vt
                    return f

                def tr_chunk(c4):
                    def f():
                        vt = vts[tb]
                        t0 = tb * QB + c4 * 128
                        b_, kt = t0 // T, (t0 % T) // KT
                        pst = psA.tile([128, 2, D], F16, tag="a", name="pst")
                        nc.tensor.transpose(
                            pst, vt[:, c4 * 128:(c4 + 1) * 128], ident)
                        nc.vector.tensor_copy(
                            out=v_sb[:, 0:2, b_, kt, 0:D], in_=pst)
                    return f

                for m in range(3):
                    chunks.append(m_chunk(m))
                for c4 in range(QB // 128):
                    chunks.append(tr_chunk(c4))
                return chunks

            # ---------- attention chunks for one (b, qb) ----------
            def attn_chunks(b_, qb):
                q0 = b_ * T + qb * QB
                n_kt = (qb + 1) * (QB // KT)
                pvs = {}
                pts = {}
                aus = {}
                dns = {}
                chunks = []

                def pv_tile(h):
                    def f():
                        pvs[h] = psPV.tile([D + 1, QB], F32, tag="pv", name="pv")
                    return f

                def sc_pair(h, p):
                    def f():
                        hs = slice(h * 64, (h + 1) * 64)
                        ps = psS.tile([128, 2, QB], F32, tag="s", name="pss")
                        for j in range(2):
                            kt = 2 * p + j
                            nc.tensor.matmul(
                                ps[:, j, :],
                                lhsT=kTt[hs, b_ * T + kt * KT:
                                         b_ * T + (kt + 1) * KT],
                                rhs=qT[hs, q0:q0 + QB],
                                start=True, stop=True)
                        pt = ptp.tile([128, 2, QB], F16, tag="pt", name="pt")
                        nc.scalar.activation(
                            out=pt, in_=ps, func=AF.Exp, scale=SCALE)
                        pts[(h, p)] = pt
                    return f

                def pv_pair(h, p):
                    def f():
                        pt = pts.pop((h, p))
                        for j in range(2):
                            kt = 2 * p + j
                            nc.tensor.matmul(
                                pvs[h],
                                lhsT=v_sb[:, h, b_, kt, :],
                                rhs=pt[:, j, :],
                                start=(kt == 0), stop=False)
                    return f

                def sc_diag(h, dp):
                    def f():
                        hs = slice(h * 64, (h + 1) * 64)
                        r0 = 2 * dp
                        off0 = r0 * KT
                        ps = psS.tile([128, 2, QB], F32, tag="s", name="psd")
                        for j in range(2):
                            r = r0 + j
                            kt = qb * (QB // KT) + r
                            off = r * KT
                            nc.tensor.matmul(
                                ps[:, j, off:QB],
                                lhsT=kTt[hs, b_ * T + kt * KT:
                                         b_ * T + (kt + 1) * KT],
                                rhs=qT[hs, q0 + off:q0 + QB],
                                start=True, stop=True)
                        pt = ptp.tile([128, 2, QB], F16, tag="pt", name="ptd")
                        nc.scalar.activation(
                            out=pt[:, :, off0:QB], in_=ps[:, :, off0:QB],
                            func=AF.Exp, scale=SCALE)
                        # causal mask: the two 128-wide triangle blocks
                        for j in range(2):
                            off = (r0 + j) * KT
                            tri = pt[:, j, off:off + KT]
                            nc.gpsimd.affine_select(
                                out=tri, in_=tri,
                                compare_op=mybir.AluOpType.is_ge, fill=0.0,
                                base=0, channel_multiplier=-1,
                                pattern=[[1, KT]])
                        pts[(h, 'd', dp)] = pt
                    return f

                def pv_diag(h, dp):
                    def f():
                        pt = pts.pop((h, 'd', dp))
                        for j in range(2):
                            r = 2 * dp + j
                            kt = qb * (QB // KT) + r
                            off = r * KT
                            nc.tensor.matmul(
                                pvs[h][:, off:QB],
                                lhsT=v_sb[:, h, b_, kt, :],
                                rhs=pt[:, j, off:QB],
                                start=(kt == 0), stop=(kt == n_kt - 1))
                    return f

                def drain(h):
                    # copy pv psum (attn rows fp16, den row fp32) to SBUF
                    # immediately so the PSUM slot frees for the next step.
                    def f():
                        au = rbp.tile([D, QB], F16, tag=f"au{h}",
                                      name="au", bufs=2)
                        nc.vector.tensor_copy(out=au, in_=pvs[h][0:D, :])
                        dn = rbp.tile([D + 1, QB], F32, tag=f"dn{h}",
                                      name="dn", bufs=2)
                        nc.vector.tensor_copy(
                            out=dn[D:D + 1, :], in_=pvs[h][D:D + 1, :])
                        aus[h] = au
                        dns[h] = dn
                    return f

                def norm():
                    # phase a (now): move den row to partition 0, recip,
                    # broadcast, and stage unnormalized h1 into attnTc.
                    # phase b (deferred to the next step, via pending_muls):
                    # the two DVE muls — by then the broadcast is complete,
                    # so they never head-of-line-block PSUM drains.
                    cols = slice(q0, q0 + QB)
                    rbs = {}
                    for h in range(HPC):
                        r0 = rbp.tile([1, QB], F32, tag="r0", name="r0")
                        nc.gpsimd.dma_start(out=r0, in_=dns[h][D:D + 1, :])
                        rc = rbp.tile([1, QB], F32, tag="rc", name="rc")
                        nc.vector.reciprocal_approx_fast(out=rc, in_=r0)
                        rb = rbp.tile([128, QB], F32, tag="rb", name="rb")
                        nc.gpsimd.partition_broadcast(rb, rc)
                        rbs[h] = rb
                    nc.gpsimd.dma_start(
                        out=attnTc[64:128, cols], in_=aus[1])

                    def muls():
                        nc.vector.tensor_mul(
                            attnTc[0:64, cols], aus[0], rbs[0][0:64, :])
                        nc.vector.tensor_mul(
                            attnTc[64:128, cols], attnTc[64:128, cols],
                            rbs[1][64:128, :])
                    pending_muls.append(muls)

                chunks.append((pv_tile(0), False))
                chunks.append((pv_tile(1), False))
                for p in range(2 * qb):            # full kt pairs
                    chunks.append((sc_pair(0, p), False))
                    chunks.append((sc_pair(1, p), True))
                    chunks.append((pv_pair(0, p), False))
                    chunks.append((pv_pair(1, p), True))
                for dp in range(2):                # diagonal pairs
                    chunks.append((sc_diag(0, dp), False))
                    chunks.append((sc_diag(1, dp), True))
                    chunks.append((pv_diag(0, dp), False))
                    chunks.append((pv_diag(1, dp), True))
                chunks.append((drain(0), False))
                chunks.append((drain(1), False))
                chunks.append((norm, False))
                return chunks

            # ---------- output projection chunks for one (b, qb) ----------
            def outproj_chunks(s):
                chunks = []

                def op_chunk(tt, ch):
                    def f():
                        t0 = s * QB + tt * 128
                        po = psA.tile([128, QB], F32, tag="a", name="po")
                        nc.tensor.matmul(
                            po,
                            lhsT=attnTc[:, t0:t0 + 128],
                            rhs=wout_sb[:, ch * QB:(ch + 1) * QB],
                            start=True, stop=True)
                        ob = obp.tile([128, QB], F16, tag="ob", name="ob")
                        # split the PSUM drains across ACT and DVE
                        if (tt + ch) % 2 == 0:
                            nc.vector.tensor_copy(out=ob, in_=po)
                        else:
                            nc.scalar.copy(out=ob, in_=po)
                        nc.sync.dma_start(
                            out=outp[t0:t0 + 128, ch * QB:(ch + 1) * QB],
                            in_=ob)
                    return f

                for tt in range(QB // 128):
                    for ch in range(2):
                        chunks.append(op_chunk(tt, ch))
                return chunks

            # ---------- software-pipelined emission ----------
            issue_x(0)
            issue_x(1)
            pending_muls = []
            for it in range(NTB + 1):
                if it + 2 < NTB:
                    issue_x(it + 2)
                # proj filler goes into the FIRST part of the attention
                # stream; outproj filler into the LAST part.
                fillA = proj_chunks(it) if it < NTB else []
                fillB = outproj_chunks(it - 2) if it >= 2 else []
                if 1 <= it:
                    s = it - 1
                    att = attn_chunks(s // NQB, s % NQB)
                    natt = len(att)
                    nA, nB = len(fillA), len(fillB)
                    cut = (natt * 11) // 20
                    fa = fb = 0
                    for i, (ck, fill_ok) in enumerate(att):
                        ck()
                        if i == min(4, natt - 2) and pending_muls:
                            for mf in pending_muls:
                                mf()
                            pending_muls.clear()
                        if i < cut:
                            wantA = (nA * (i + 1) + cut - 1) // cut
                            while fa < min(wantA, nA):
                                fillA[fa]()
                                fa += 1
                        else:
                            wantB = (nB * (i + 1 - cut) + natt - cut - 1) \
                                // (natt - cut)
                            while fb < min(wantB, nB):
                                fillB[fb]()
                                fb += 1
                    while fa < nA:
                        fillA[fa]()
                        fa += 1
                    while fb < nB:
                        fillB[fb]()
                        fb += 1
                    if it == NTB:
                        # drain the final block's normalize + out-projection
                        for mf in pending_muls:
                            mf()
                        pending_muls.clear()
                        for ck in outproj_chunks(it - 1):
                            ck()
                else:
                    for ck in fillA + fillB:
                        ck()
    nc.compile()
    return nc


def _prep_in_maps(x, w_qkv, b_qkv, w_out):
    xTfull = np.ascontiguousarray(x.reshape(BT, C).T.astype(np.float16))
    in_maps = []
    for core in range(NCORES):
        hs = [core * HPC + i for i in range(HPC)]
        wq = np.ascontiguousarray(np.concatenate(
            [w_qkv[:, sec * C + h * D: sec * C + (h + 1) * D]
             for sec in range(3) for h in hs], axis=1).astype(np.float16))
        bq = np.ascontiguousarray(np.stack(
            [np.concatenate([b_qkv[sec * C + h * D: sec * C + (h + 1) * D] for h in hs])
             for sec in range(3)], axis=1))
        wo = np.ascontiguousarray(np.concatenate(
            [w_out[h * D:(h + 1) * D, :] for h in hs], axis=0).astype(np.float16))
        in_maps.append({"xT": xTfull, "wqkv": wq, "bqkv": bq, "wout": wo})
    return in_maps


LAST_RESULTS = None


def kernel(x, w_qkv, b_qkv, w_out, b_out):
    global LAST_RESULTS
    x = np.asarray(x, np.float32)
    w_qkv = np.asarray(w_qkv, np.float32)
    b_qkv = np.asarray(b_qkv, np.float32)
    w_out = np.asarray(w_out, np.float32)
    b_out = np.asarray(b_out, np.float32)

    if "nc" not in _cache:
        _cache["nc"] = _build_bass()
    nc = _cache["nc"]

    in_maps = _prep_in_maps(x, w_qkv, b_qkv, w_out)
    res = bass_utils.run_bass_kernel_spmd(nc, in_maps, core_ids=list(range(NCORES)))
    LAST_RESULTS = res

    out = res.results[0]["outp"].astype(np.float32)
    for r_ in res.results[1:]:
        out += r_["outp"].astype(np.float32)
    out += b_out
    return out.reshape(B, T, C)
